# revision 1
# baseline (speedup 1.0000x reference)
import numpy as np

# nn_CascadedGroupAttention: B=1024, DIM=256, RES=14, heads=4, KEY_DIM=16, D=64
# Data-parallel over batch across 8 NeuronCores (sharding_hint), params replicated.
KEY_DIM = 16
NUM_HEADS = 4
D = 64
RES = 14
DIM = 256
B = 1024
SCALE = KEY_DIM ** -0.5
EPS = 1e-5
NCORES = 8


def _forward_shard(x, qkv_w, qkv_b, dw_w, dw_b, proj_w, bn_gamma, bn_beta,
                   bn_mean, bn_var, attn_biases, bias_idxs):
    import jax, jax.numpy as jnp
    from jax import lax
    Bsz, C, H, W = x.shape
    N = H * W
    ab = attn_biases[:, bias_idxs]
    feats_in = jnp.split(x, NUM_HEADS, axis=1)
    feats_out = []
    feat = feats_in[0]
    for i in range(NUM_HEADS):
        if i > 0:
            feat = feat + feats_in[i]
        f = jnp.einsum('oc,bchw->bohw', qkv_w[i], feat) + qkv_b[i][None, :, None, None]
        q = f[:, :KEY_DIM]
        k = f[:, KEY_DIM:2 * KEY_DIM]
        v = f[:, 2 * KEY_DIM:]
        q = lax.conv_general_dilated(q, dw_w[i], (1, 1), 'SAME',
                                     dimension_numbers=('NCHW', 'OIHW', 'NCHW'),
                                     feature_group_count=KEY_DIM)
        q = q + dw_b[i][None, :, None, None]
        qf = q.reshape(Bsz, KEY_DIM, N)
        kf = k.reshape(Bsz, KEY_DIM, N)
        vf = v.reshape(Bsz, D, N)
        attn = jnp.einsum('bcn,bcm->bnm', qf, kf) * SCALE + ab[i]
        attn = jax.nn.softmax(attn, axis=-1)
        feat = jnp.einsum('bdm,bnm->bdn', vf, attn).reshape(Bsz, D, H, W)
        feats_out.append(feat)
    cat = jnp.concatenate(feats_out, axis=1)
    h = jax.nn.relu(cat)
    h = jnp.einsum('oc,bchw->bohw', proj_w, h)
    inv = bn_gamma / jnp.sqrt(bn_var + EPS)
    return h * inv[None, :, None, None] + (bn_beta - bn_mean * inv)[None, :, None, None]


def kernel(**inputs) -> np.ndarray:
    import jax
    devs = jax.devices()[:NCORES]
    x = np.asarray(inputs["x"])
    xs = x.reshape(NCORES, B // NCORES, DIM, RES, RES)
    params = {k: np.asarray(v) for k, v in inputs.items() if k != "x"}
    # replicate params, shard x
    pm = jax.pmap(lambda xs_, p: _forward_shard(xs_, **p), devices=devs)
    pr = {k: np.broadcast_to(v, (NCORES,) + v.shape) for k, v in params.items()}
    out = pm(xs, pr)
    return np.asarray(out).reshape(B, DIM, RES, RES)


# revision 2
# speedup vs baseline: 2.5884x; 2.5884x over previous
import numpy as np

# nn_CascadedGroupAttention_75453985457396
# B=1024, DIM=256, RES=14 (N=196), heads=4, KEY_DIM=16, D=64.
# Pure data parallel over batch across the 8 NeuronCores; all conv/attention
# parameters are replicated (tiny). Per-head cascade stays sequential.
KEY_DIM = 16
NUM_HEADS = 4
D = 64
RES = 14
DIM = 256
B = 1024
SCALE = KEY_DIM ** -0.5
EPS = 1e-5
NCORES = 8


def _forward_shard(x, qkv_w, qkv_b, dw_w, dw_b, proj_w, bn_scale, bn_shift, eab):
    # eab = exp(attn_biases[:, bias_idxs]) precomputed host-side (tiny).
    # Softmax without max-subtraction: logits here are O(1) so exp() is safe;
    # normalization is applied after the AV contraction on the small [B,D,N]
    # tensor instead of the [B,N,N] one (saves full N^2 memory passes).
    import jax, jax.numpy as jnp
    from jax import lax
    Bsz, C, H, W = x.shape
    N = H * W
    feats_in = jnp.split(x, NUM_HEADS, axis=1)
    feats_out = []
    feat = feats_in[0]
    for i in range(NUM_HEADS):
        if i > 0:
            feat = feat + feats_in[i]
        f = jnp.einsum('oc,bchw->bohw', qkv_w[i], feat) + qkv_b[i][None, :, None, None]
        q = f[:, :KEY_DIM]
        k = f[:, KEY_DIM:2 * KEY_DIM]
        v = f[:, 2 * KEY_DIM:]
        q = lax.conv_general_dilated(q, dw_w[i], (1, 1), 'SAME',
                                     dimension_numbers=('NCHW', 'OIHW', 'NCHW'),
                                     feature_group_count=KEY_DIM)
        q = q + dw_b[i][None, :, None, None]
        qf = q.reshape(Bsz, KEY_DIM, N)
        kf = k.reshape(Bsz, KEY_DIM, N)
        vf = v.reshape(Bsz, D, N)
        e = jnp.exp(jnp.einsum('bcn,bcm->bnm', qf, kf) * SCALE) * eab[i][None]
        s = jnp.sum(e, axis=-1)                       # [B, N]
        o = jnp.einsum('bdm,bnm->bdn', vf, e)         # unnormalized AV
        feat = (o / s[:, None, :]).reshape(Bsz, D, H, W)
        feats_out.append(feat)
    cat = jnp.concatenate(feats_out, axis=1)
    h = jax.nn.relu(cat)
    h = jnp.einsum('oc,bchw->bohw', proj_w, h)
    return h * bn_scale[None, :, None, None] + bn_shift[None, :, None, None]


_CACHE = {}


def _get_pm():
    import jax
    if "pm" not in _CACHE:
        devs = jax.devices()[:NCORES]
        _CACHE["pm"] = jax.pmap(
            lambda xs_, p: _forward_shard(xs_, **p), devices=devs)
    return _CACHE["pm"]


def kernel(**inputs) -> np.ndarray:
    import jax
    x = np.asarray(inputs["x"], dtype=np.float32)
    xs = x.reshape(NCORES, B // NCORES, DIM, RES, RES)

    # host-side prep of tiny replicated parameters
    ab = np.asarray(inputs["attn_biases"], np.float32)[
        :, np.asarray(inputs["bias_idxs"])]          # [heads, N, N]
    eab = np.exp(ab.astype(np.float64)).astype(np.float32)
    inv = (np.asarray(inputs["bn_gamma"], np.float32)
           / np.sqrt(np.asarray(inputs["bn_var"], np.float32) + EPS))
    params = dict(
        qkv_w=np.asarray(inputs["qkv_w"], np.float32),
        qkv_b=np.asarray(inputs["qkv_b"], np.float32),
        dw_w=np.asarray(inputs["dw_w"], np.float32),
        dw_b=np.asarray(inputs["dw_b"], np.float32),
        proj_w=np.asarray(inputs["proj_w"], np.float32),
        bn_scale=inv,
        bn_shift=(np.asarray(inputs["bn_beta"], np.float32)
                  - np.asarray(inputs["bn_mean"], np.float32) * inv),
        eab=eab,
    )
    pm = _get_pm()
    pr = {k: np.broadcast_to(v, (NCORES,) + v.shape) for k, v in params.items()}
    out = pm(xs, pr)
    return np.asarray(out).reshape(B, DIM, RES, RES)
